# revision 29
# baseline (speedup 1.0000x reference)
"""Trainium2 Bass kernel for nn_ExportGatedDeltaNet (gated linear attention
with depthwise conv, chunked recurrence).

Self-contained: hardcodes shapes/sharding. Sharding: 8-way tensor-parallel
over heads (each core owns 4 of the 32 value heads / 2 of the 16 key heads);
both batch elements are processed on every core (slots interleave the two
batches). Each core computes a full [B, T, C] partial of the output
projection over its head slice; the host sums the 8 partials.

v2 redesign vs baseline:
- q is left unnormalized: the gated RMSNorm downstream is invariant to
  per-timestep scaling of the attention output, so q/|q| is unnecessary.
- k is normalized in transposed space (per-partition row scale after the
  PE transpose) -- no broadcast matmuls, no DVE reciprocal.
- rsqrt/log-sigmoid/softplus are computed as exp/ln pairs so every
  non-silu activation lives in the single natural_log_exp table
  (2 act-table loads per slot instead of ~14).
- DMA transposes (1.2us each on SP) replaced by PE transposes + Pool copies.
- Emission interleaves slot s's recurrence (phase B/C) with slot s+1's
  projection/conv/decay precompute (phase A) and alternates batch elements
  between consecutive slots, keeping the PE stream dense (p-state ramp).
"""

import numpy as np
import ml_dtypes

import concourse.bass as bass
import concourse.tile as tile
from concourse import mybir
from concourse.vector_clock import ScopedClock, VectorClock
from concourse.bass_utils import run_bass_kernel_spmd

F32 = mybir.dt.float32
F32R = mybir.dt.float32r
BF16 = mybir.dt.bfloat16
AF = mybir.ActivationFunctionType
OP = mybir.AluOpType
BF16_NP = ml_dtypes.bfloat16

NK, NV, DK, DV, KCONV, C = 16, 32, 128, 128, 4, 2048
KEY = NK * DK            # 2048
B, T = 2, 2048
L = 128                  # recurrence chunk length
TB = 512                 # t-block (slot length)
NTB = T // TB            # 4
NCH = TB // L            # chunks per slot
NCORES = 8

# per-core head slice
VH = NV // NCORES        # 4 value heads
KH = NK // NCORES        # 2 key heads
QCH = KH * DK            # 256
VCH = VH * DV            # 512
ZCH = VH * DV            # 512
CONVCH = 2 * QCH + VCH   # 1024 channels through the conv
TOTCH = CONVCH + ZCH + 32 + VH  # 1572: ..., b(4), pad(28), a(4)
N_CT = C // 128          # 16 contraction tiles
N_CONVT = CONVCH // 128  # 8
N_ZT = ZCH // 128        # 4
N_WT = TOTCH // 128      # 12 full tiles + 36 extra cols handled separately


def _walrus_safe_drain(self, tick_clock, wait_clock):
    # The container's walrus rejects >1 sync-wait on CTRL-class instructions;
    # split the final drain's waits across single-wait nops.
    vals = eval(repr(tick_clock.global_clock).replace("VectorClock", ""))
    for j, v in enumerate(vals):
        if not v:
            continue
        masked = [0] * len(vals)
        masked[j] = v
        nop_inst = self.nc.sync.nop(nofuse=True)
        wait_clock.add_sem_waits(
            nop_inst.ins, ScopedClock({None: VectorClock(masked)})
        )
    self.nc.sync.drain()
    self.nc.all_engine_barrier()
    popped = self.nc._tile_sem_poison_stack.pop()
    assert popped is self._sem_poison
    self.nc.clear_and_free_semaphores(list(self.sems.allocated().values()))
    self.nc.all_engine_barrier()


tile.TileContext._drain_and_barrier = _walrus_safe_drain


# The container's walrus rejects >1 sync-wait on any instruction. Tile's
# semaphore pass emits multi-wait instructions, so split them at the BIR-JSON
# level: hoist all but one wait onto NoOps (same engine) inserted just before.
_orig_to_json_bytes = bass.Bass.to_json_bytes
_WSPLIT = [0]


def _split_multi_waits(self, *args, **kwargs):
    import json
    raw = _orig_to_json_bytes(self, *args, **kwargs)
    m = json.loads(raw)
    changed = False
    for f in m["functions"]:
        for bb in f["blocks"]:
            out_insts = []
            for inst in bb["instructions"]:
                si = inst.get("sync_info")
                waits = (si or {}).get("on_wait") or []
                if len(waits) > 1:
                    changed = True
                    for w in waits[:-1]:
                        _WSPLIT[0] += 1
                        out_insts.append({
                            "debug": inst.get("debug"),
                            "engine": inst["engine"],
                            "ins": [], "outs": [],
                            "name": f"I-wsplit-{_WSPLIT[0]}",
                            "opcode": "NoOp",
                            "sync_info": {"on_update": [], "on_wait": [w]},
                        })
                    si["on_wait"] = [waits[-1]]
                out_insts.append(inst)
            bb["instructions"] = out_insts
    if not changed:
        return raw
    return json.dumps(m).encode()


bass.Bass.to_json_bytes = _split_multi_waits

# HWDGE DMAs execute on DMA-queue timelines, where a hoisted same-engine NoOp
# wait does not gate them. Route static DMAs through the SP sequencer instead
# so program order (and the NoOp wait splitting) applies to them too.
import concourse.bass_utils as _bu

_orig_run_command = _bu.run_command


def _patched_run_command(argv, **kwargs):
    argv = [a.replace("--assign-static-dmas-to-sp=false",
                      "--assign-static-dmas-to-sp=true")
            for a in argv]
    return _orig_run_command(argv, **kwargs)


_bu.run_command = _patched_run_command


def build_kernel():
    nc = bass.Bass(num_swdge_queues=4)

    xt = nc.dram_tensor("xt", [B, C, T], BF16, kind="ExternalInput")
    wt = nc.dram_tensor("wt", [C, TOTCH], BF16, kind="ExternalInput")
    wout = nc.dram_tensor("wout", [VCH, C], BF16, kind="ExternalInput")
    convw = nc.dram_tensor("convw", [128, N_CONVT, KCONV], F32,
                           kind="ExternalInput")
    halo = nc.dram_tensor("halo", [B, 128, N_CONVT, KCONV - 1], BF16,
                          kind="ExternalInput")
    s0 = nc.dram_tensor("s0", [DK, B, VH, DV], F32, kind="ExternalInput")
    dtb = nc.dram_tensor("dtb", [VH, 1], F32, kind="ExternalInput")
    nega = nc.dram_tensor("nega", [VH, 1], F32, kind="ExternalInput")
    normw = nc.dram_tensor("normw", [128, 1], F32, kind="ExternalInput")
    out = nc.dram_tensor("out", [B, T, C], BF16, kind="ExternalOutput")

    # constants embedded in the NEFF
    ut_np = np.triu(np.ones((L, L), np.float32))              # [u,t]: u<=t
    UT = nc.inline_tensor(ut_np.astype(BF16_NP), name="UT")
    STA = nc.inline_tensor((1.0 - ut_np).astype(BF16_NP), name="STA")  # u>t'
    ONESM = nc.inline_tensor(np.ones((L, L), BF16_NP), name="ONESM")
    NEGM = nc.inline_tensor(
        np.where(ut_np > 0, 0.0, -1e30).astype(np.float32), name="NEGM")
    IDENT128 = nc.inline_tensor(np.eye(128, dtype=BF16_NP), name="IDENT128")
    ONES_COL = nc.inline_tensor(np.ones((128, 1), BF16_NP), name="ONES_COL")
    ONE_F = nc.inline_tensor(np.ones((128, 1), np.float32), name="ONE_F")
    EPS12 = nc.inline_tensor(np.full((128, 1), 1e-12, np.float32), name="EPS12")
    EPS6 = nc.inline_tensor(np.full((128, 1), 1e-6, np.float32), name="EPS6")
    ONES_ROW = nc.inline_tensor(np.ones((1, 128), np.float32), name="ONES_ROW")

    from contextlib import ExitStack
    with nc.allow_low_precision(reason="bf16 compute by design"), \
         tile.TileContext(nc) as tc, ExitStack() as stack:
        consts = stack.enter_context(tc.tile_pool(name="consts", bufs=1))
        wpool = stack.enter_context(tc.tile_pool(name="wpool", bufs=1))
        statep = stack.enter_context(tc.tile_pool(name="statep", bufs=1))
        xpool = stack.enter_context(tc.tile_pool(name="xpool", bufs=2))
        rawp = stack.enter_context(tc.tile_pool(name="rawp", bufs=2))
        projp = stack.enter_context(tc.tile_pool(name="projp", bufs=1))
        decp = stack.enter_context(tc.tile_pool(name="decp", bufs=2))
        ogp = stack.enter_context(tc.tile_pool(name="ogp", bufs=2))
        outp = stack.enter_context(tc.tile_pool(name="outp", bufs=2))
        pA = stack.enter_context(tc.tile_pool(name="pA", bufs=2, space="PSUM"))
        pTR = stack.enter_context(tc.tile_pool(name="pTR", bufs=2, space="PSUM"))
        pB = stack.enter_context(tc.tile_pool(name="pB", bufs=2, space="PSUM"))
        pC = stack.enter_context(tc.tile_pool(name="pC", bufs=2, space="PSUM"))

        # ---- load constants ----
        ut_sb = consts.tile([L, L], BF16)
        nc.gpsimd.dma_start(ut_sb[:], UT[:])
        sta_sb = consts.tile([L, L], BF16)
        nc.gpsimd.dma_start(sta_sb[:], STA[:])
        onesm_sb = consts.tile([L, L], BF16)
        nc.gpsimd.dma_start(onesm_sb[:], ONESM[:])
        negm_sb = consts.tile([L, L], F32)
        nc.gpsimd.dma_start(negm_sb[:], NEGM[:])
        ident128_sb = consts.tile([128, 128], BF16)
        nc.gpsimd.dma_start(ident128_sb[:], IDENT128[:])
        onescol_sb = consts.tile([128, 1], BF16)
        nc.gpsimd.dma_start(onescol_sb[:], ONES_COL[:])
        one_sb = consts.tile([128, 1], F32)
        nc.gpsimd.dma_start(one_sb[:], ONE_F[:])
        eps12_sb = consts.tile([128, 1], F32)
        nc.gpsimd.dma_start(eps12_sb[:], EPS12[:])
        eps6_sb = consts.tile([128, 1], F32)
        nc.gpsimd.dma_start(eps6_sb[:], EPS6[:])
        onesrow_sb = consts.tile([1, 128], F32R)
        nc.gpsimd.dma_start(onesrow_sb[:], ONES_ROW[:].bitcast(F32R))
        convw_sb = consts.tile([128, N_CONVT, KCONV], F32)
        nc.gpsimd.dma_start(convw_sb[:], convw[:])
        dtb_sb = consts.tile([VH, 1], F32)
        nc.gpsimd.dma_start(dtb_sb[:], dtb[:])
        nega_sb = consts.tile([VH, 1], F32)
        nc.gpsimd.dma_start(nega_sb[:], nega[:])
        normw_sb = consts.tile([128, 1], F32)
        nc.gpsimd.dma_start(normw_sb[:], normw[:])

        # resident weights (xt0 is loaded first, below, so proj can start)
        wt_sb = wpool.tile([128, N_CT, TOTCH], BF16)
        wout_sb = wpool.tile([128, VH, C], BF16)

        # recurrent state (f32 truth + bf16 matmul copy) and conv halo
        S = statep.tile([128, B, VH, DV], F32, tag="S")
        nc.gpsimd.dma_start(S[:], s0[:])
        Sb = statep.tile([128, B, VH, DV], BF16, tag="Sb")
        nc.vector.tensor_copy(Sb[:], S[:])
        halo_sb = statep.tile([128, B, N_CONVT, KCONV - 1], BF16, tag="halo")
        nc.gpsimd.dma_start(halo_sb[:], halo.rearrange("b p k j -> p b k j"))

        # ---- per-slot persistent-ish tiles are allocated inside units ----
        slots = [(i % 2, i // 2) for i in range(B * NTB)]  # (b, tb) interleaved
        NSLOT = len(slots)

        # cross-unit state passed via dicts keyed by slot index
        ST = {i: {} for i in range(NSLOT)}

        # slot 0's x is loaded up front
        xt0 = xpool.tile([128, N_CT, TB], BF16, tag="xt")
        ST[0]["xt"] = xt0
        b0, tb0 = slots[0]
        nc.gpsimd.dma_start(
            xt0[:],
            xt[b0].rearrange("(ko p) t -> p ko t", p=128)
            [:, :, tb0 * TB:(tb0 + 1) * TB])
        wt_r = wt.rearrange("(ko p) f -> p ko f", p=128)
        for c0, c1 in [(0, 128), (128, 384), (384, 768), (768, 1152),
                       (1152, TOTCH)]:
            nc.gpsimd.dma_start(wt_sb[:, :, c0:c1], wt_r[:, :, c0:c1])
        nc.gpsimd.dma_start(wout_sb[:],
                            wout.rearrange("(vo p) f -> p vo f", p=128))

        def emit_A_units(i):
            """Phase A for slot i: projection, conv, decay precompute.
            Returns a list of callables."""
            b, tb = slots[i]
            tsl = slice(tb * TB, (tb + 1) * TB)
            st = ST[i]
            units = []

            def u_prefetch():
                # prefetch x for the *next* slot; A(i) is emitted one step
                # early, so this lands ~1.5 steps before the data is consumed.
                if i + 1 < NSLOT:
                    nb_, ntb_ = slots[i + 1]
                    xt_t = xpool.tile([128, N_CT, TB], BF16, tag="xt")
                    ST[i + 1]["xt"] = xt_t
                    nc.gpsimd.dma_start(
                        xt_t[:],
                        xt[nb_].rearrange("(ko p) t -> p ko t", p=128)
                        [:, :, ntb_ * TB:(ntb_ + 1) * TB])
            units.append(u_prefetch)

            def u_rawinit():
                raw = rawp.tile([128, N_CONVT, TB + KCONV - 1], BF16, tag="raw")
                st["raw"] = raw
                nc.gpsimd.tensor_copy(raw[:, :, 0:KCONV - 1], halo_sb[:, b])
                st["z"] = projp.tile([128, N_ZT, TB], BF16, tag="z", bufs=2,
                                     name="z")
                st["qf"] = projp.tile([128, KH, TB], BF16, tag="qf", name="qf")
                st["kf"] = projp.tile([128, KH, TB], BF16, tag="kf", name="kf")
                st["vf"] = projp.tile([128, VH, TB], BF16, tag="vf", name="vf")
            units.append(u_rawinit)

            def u_proj(cht):
                ps = pA.tile([128, TB], F32, tag="A")
                for ct in range(N_CT):
                    nc.tensor.matmul(
                        ps[:], wt_sb[:, ct, cht * 128:(cht + 1) * 128],
                        st["xt"][:, ct, :],
                        start=(ct == 0), stop=(ct == N_CT - 1))
                if cht < N_CONVT:
                    if cht % 2 == 0:
                        nc.vector.tensor_copy(
                            st["raw"][:, cht, KCONV - 1:TB + KCONV - 1], ps[:])
                    else:
                        nc.scalar.activation(
                            st["raw"][:, cht, KCONV - 1:TB + KCONV - 1], ps[:],
                            AF.Copy)
                else:
                    zi = cht - N_CONVT
                    nc.scalar.activation(st["z"][:, zi, :], ps[:], AF.Silu)
            for cht in range(N_WT):
                units.append(lambda cht=cht: u_proj(cht))

            def u_halo_save():
                # stash the last 3 input positions for this batch's next slot
                nc.gpsimd.tensor_copy(
                    halo_sb[:, b], st["raw"][:, :, TB:TB + KCONV - 1])
            units.append(u_halo_save)

            def u_ba():
                ps = pA.tile([128, TB], F32, tag="A")
                for ct in range(N_CT):
                    nc.tensor.matmul(
                        ps[0:32 + VH, :], wt_sb[:, ct, N_WT * 128:TOTCH],
                        st["xt"][:, ct, :],
                        start=(ct == 0), stop=(ct == N_CT - 1))
                gsp = projp.tile([VH, 2, TB], BF16, tag="gsp")
                st["gsp"] = gsp
                tmp = projp.tile([VH, TB], BF16, tag="batmp", bufs=2)
                # lnb' = -ln(sigmoid(b)) = ln(1 + exp(-b))
                nc.scalar.activation(tmp[:], ps[0:VH, :], AF.Exp, scale=-1.0)
                nc.scalar.activation(gsp[:, 1, :], tmp[:], AF.Ln, bias=one_sb[0:VH])
                # softplus(a + dtb) = ln(1 + exp(a + dtb))
                tmp2 = projp.tile([VH, TB], BF16, tag="batmp", bufs=2)
                nc.scalar.activation(tmp2[:], ps[32:32 + VH, :], AF.Exp,
                                     bias=dtb_sb[:])
                tmp3 = projp.tile([VH, TB], BF16, tag="batmp2", bufs=2)
                nc.scalar.activation(tmp3[:], tmp2[:], AF.Ln, bias=one_sb[0:VH])
                # g = -exp(A_log) * softplus
                nc.vector.tensor_scalar(gsp[:, 0, :], tmp3[:], nega_sb[:],
                                        None, OP.mult)
            units.append(u_ba)

            def u_conv(cht):
                acc = projp.tile([128, TB], BF16, tag="convacc", bufs=2)
                nc.vector.tensor_scalar(
                    acc[:], st["raw"][:, cht, 0:TB], convw_sb[:, cht, 0:1],
                    None, OP.mult)
                for j in range(1, KCONV):
                    nc.vector.scalar_tensor_tensor(
                        acc[:], st["raw"][:, cht, j:TB + j],
                        convw_sb[:, cht, j:j + 1], acc[:],
                        OP.mult, OP.add)
                if cht < KH:
                    nc.scalar.activation(st["qf"][:, cht, :], acc[:], AF.Silu)
                elif cht < 2 * KH:
                    nc.scalar.activation(st["kf"][:, cht - KH, :], acc[:],
                                         AF.Silu)
                else:
                    nc.scalar.activation(st["vf"][:, cht - 2 * KH, :], acc[:],
                                         AF.Silu)
            for cht in range(N_CONVT):
                units.append(lambda cht=cht: u_conv(cht))

            def u_qsq():
                # ssq of raw q per timestep, for exact rmsnorm eps compensation
                qsq = decp.tile([1, KH, TB], BF16, tag="qsq", name="qsq")
                st["qsq"] = qsq
                for kh in range(KH):
                    sqq = ogp.tile([128, TB], BF16, tag="sq", bufs=2)
                    nc.vector.tensor_tensor(sqq[:], st["qf"][:, kh, :],
                                            st["qf"][:, kh, :], OP.mult)
                    qsp = pC.tile([1, TB], F32, tag="C")
                    nc.tensor.matmul(qsp[:], onescol_sb[:], sqq[:],
                                     start=True, stop=True)
                    nc.scalar.activation(qsq[0:1, kh, :], qsp[:], AF.Copy)
            units.append(u_qsq)

            def u_predecl():
                st["gspT"] = decp.tile([128, NCH, 2 * VH], F32, tag="gspT",
                                       bufs=1, name="gspT")
                st["ktT"] = decp.tile([128, NCH, KH, L], BF16, tag="ktT", name="ktT")
                st["vT"] = decp.tile([128, NCH, VH, L], BF16, tag="vT", name="vT")
                st["qh"] = decp.tile([128, NCH, VH, L], BF16, tag="qh", name="qh")
                st["PT"] = decp.tile([128, NCH, VH, L], BF16, tag="PT", name="PT")
                st["ksq"] = decp.tile([128, NCH * KH], F32, tag="ksq", name="ksq")
                st["rinvk"] = decp.tile([128, NCH, KH], F32, tag="rinvk", name="rinvk")
                st["rk2"] = decp.tile([128, NCH * VH], F32, tag="rk2", name="rk2")
                st["dtlast"] = decp.tile([128, NCH * VH], F32, tag="dtlast", name="dtlast")
                st["dclast"] = decp.tile([128, NCH * VH], F32, tag="dclast", name="dclast")
            units.append(u_predecl)

            def u_chunkpre(c):
                t0 = c * L
                csl = slice(t0, t0 + L)
                # --- transposes: k, v chunks -> t-partitioned (PE + Pool) ---
                trq = pTR.tile([128, 6 * L + 2 * VH], BF16, tag="trb")
                for kh in range(KH):
                    nc.tensor.transpose(trq[:, kh * L:(kh + 1) * L],
                                        st["kf"][:, kh, csl], ident128_sb[:])
                for h in range(VH):
                    nc.tensor.transpose(
                        trq[:, (KH + h) * L:(KH + h + 1) * L],
                        st["vf"][:, h, csl], ident128_sb[:])
                nc.scalar.activation(
                    st["ktT"][:, c].rearrange("p a b -> p (a b)"),
                    trq[:, 0:KH * L], AF.Copy)
                nc.vector.tensor_copy(st["vT"][:, c],
                                      trq[:, KH * L:(KH + VH) * L])
                # --- gsp transposes ride in the same bf16 quad ---
                nc.tensor.transpose(trq[:, 6 * L:6 * L + VH],
                                    st["gsp"][0:VH, 0, csl],
                                    ident128_sb[0:VH, 0:VH])
                nc.tensor.transpose(trq[:, 6 * L + VH:6 * L + 2 * VH],
                                    st["gsp"][0:VH, 1, csl],
                                    ident128_sb[0:VH, 0:VH])
                nc.scalar.activation(st["gspT"][:, c],
                                      trq[:, 6 * L:6 * L + 2 * VH], AF.Copy)
                quad = pB.tile([128, TB], F32, tag="B")
                # --- Ball / Dps / dtps ---
                ball = decp.tile([128, VH, L], BF16, tag="ball")
                for h in range(VH):
                    nc.vector.tensor_scalar(
                        ball[:, h, :], ut_sb[:], st["gspT"][:, c, h:h + 1],
                        None, OP.mult)
                dps = pA.tile([128, VH, L], F32, tag="A")
                nc.tensor.matmul(dps[:].rearrange("p a b -> p (a b)"),
                                 sta_sb[:],
                                 ball[:].rearrange("p a b -> p (a b)"),
                                 start=True, stop=True)
                dtps = pA.tile([128, VH, L], F32, tag="A")
                nc.tensor.matmul(dtps[:].rearrange("p a b -> p (a b)"),
                                 onesm_sb[:],
                                 ball[:].rearrange("p a b -> p (a b)"),
                                 start=True, stop=True)
                # ebr = exp(cumsum g) broadcast; last col kept in f32
                ebr = decp.tile([128, VH, L], BF16, tag="ebr")
                nc.scalar.activation(ebr[:].rearrange("p a b -> p (a b)"),
                                     dtps[:].rearrange("p a b -> p (a b)"),
                                     AF.Exp)
                nc.scalar.activation(
                    st["dtlast"][:, c * VH:(c + 1) * VH],
                    dtps[:, :, L - 1:L].rearrange("p a b -> p (a b)"), AF.Exp)
                # Eall = Dps - lnb' + negm ; Decay = exp(Eall)
                escr = decp.tile([128, VH, L], BF16, tag="escr")
                for h in range(VH):
                    nc.vector.scalar_tensor_tensor(
                        escr[:, h, :], dps[:, h, :],
                        st["gspT"][:, c, VH + h:VH + h + 1], negm_sb[:],
                        OP.subtract, OP.add)
                decay = decp.tile([128, VH, L], BF16, tag="decay")
                nc.scalar.activation(decay[:].rearrange("p a b -> p (a b)"),
                                     escr[:].rearrange("p a b -> p (a b)"),
                                     AF.Exp)
                nc.scalar.activation(
                    st["dclast"][:, c * VH:(c + 1) * VH],
                    escr[:, :, L - 1:L].rearrange("p a b -> p (a b)"), AF.Exp)
                # --- k norm in transposed space ---
                kscr = decp.tile([128, L], BF16, tag="kscr")
                for kh in range(KH):
                    nc.vector.scalar_tensor_tensor(
                        kscr[:], st["ktT"][:, c, kh], 1.0,
                        st["ktT"][:, c, kh], OP.mult, OP.mult,
                        accum_out=st["ksq"][:, c * KH + kh:c * KH + kh + 1])
                nc.scalar.activation(st["rinvk"][:, c],
                                     st["ksq"][:, c * KH:(c + 1) * KH],
                                     AF.Ln, bias=eps12_sb[:])
                nc.scalar.activation(st["rinvk"][:, c], st["rinvk"][:, c],
                                     AF.Exp, scale=-0.5)
                # --- Pps (raw attention logits), PT, qh, rk2 ---
                for kh in range(KH):
                    nc.tensor.matmul(quad[:, kh * L:(kh + 1) * L],
                                     st["kf"][:, kh, csl],
                                     st["qf"][:, kh, csl],
                                     start=True, stop=True)
                for h in range(VH):
                    kh = h // (VH // KH)
                    nc.vector.scalar_tensor_tensor(
                        st["PT"][:, c, h], quad[:, kh * L:(kh + 1) * L],
                        st["rinvk"][:, c, kh:kh + 1], decay[:, h, :],
                        OP.mult, OP.mult)
                    nc.vector.tensor_tensor(
                        st["qh"][:, c, h], st["qf"][:, kh, csl], ebr[:, h, :],
                        OP.mult)
                    nc.vector.tensor_tensor(
                        st["rk2"][:, c * VH + h:c * VH + h + 1],
                        st["rinvk"][:, c, kh:kh + 1],
                        st["dclast"][:, c * VH + h:c * VH + h + 1], OP.mult)
            groups = {"head": units[0:2], "proj": units[2:2 + N_WT + 2],
                      "conv": units[2 + N_WT + 2:2 + N_WT + 2 + N_CONVT],
                      "pre": units[2 + N_WT + 2 + N_CONVT:],
                      "chunkpre": [lambda c=c: u_chunkpre(c)
                                   for c in range(NCH)]}
            return groups

        def emit_BC_units(i):
            """Phase B (recurrence) + C (rmsnorm + out-proj) for slot i.
            For the final slot, rms/outproj run per-chunk right after each
            recurrence chunk (nothing follows to hide the serial tail)."""
            b, tb = slots[i]
            tail = (i == NSLOT - 1)
            st = ST[i]
            units = []

            def u_B(c):
                og = st["og"]
                kt2 = decp.tile([128, VH, L], BF16, tag="kt2")
                for h in range(VH):
                    nc.vector.tensor_scalar(
                        kt2[:, h, :], st["ktT"][:, c, h // (VH // KH)],
                        st["rk2"][:, c * VH + h:c * VH + h + 1], None, OP.mult)
                pops = pB.tile([128, TB], F32, tag="B")
                sps = pB.tile([128, TB], F32, tag="B")
                for h in range(VH):
                    nc.tensor.matmul(pops[:, h * L:(h + 1) * L],
                                     Sb[:, b, h, :], st["qh"][:, c, h],
                                     start=True, stop=False)
                    nc.tensor.matmul(sps[:, h * L:(h + 1) * L],
                                     kt2[:, h, :], st["vT"][:, c, h],
                                     start=True, stop=True)
                    nc.tensor.matmul(pops[:, h * L:(h + 1) * L],
                                     st["vT"][:, c, h], st["PT"][:, c, h],
                                     start=False, stop=True)
                for h in range(VH):
                    nc.vector.tensor_tensor(
                        og[:, h, c * L:(c + 1) * L],
                        pops[:, h * L:(h + 1) * L],
                        st["z"][:, h, c * L:(c + 1) * L], OP.mult)
                for h in range(VH):
                    nc.vector.scalar_tensor_tensor(
                        S[:, b, h, :], S[:, b, h, :],
                        st["dtlast"][:, c * VH + h:c * VH + h + 1],
                        sps[:, h * L:(h + 1) * L], OP.mult, OP.add)
                nc.scalar.activation(
                    Sb[:, b].rearrange("p a b -> p (a b)"),
                    S[:, b].rearrange("p a b -> p (a b)"), AF.Copy)

            def u_B0(c=0):
                st["og"] = ogp.tile([128, VH, TB], BF16, tag="og", name="og")
                u_B(0)
            units.append(u_B0)
            for c in range(1, NCH):
                units.append(lambda c=c: u_B(c))

            groups = {"B": list(units), "rms": [], "outproj": []}

            def u_rms(h):
                if h == 0:
                    st["ogn"] = ogp.tile([128, VH, TB], BF16, tag="ogn",
                                         bufs=1, name="ogn")
                og = st["og"]
                sq = ogp.tile([128, TB], BF16, tag="sq", bufs=2)
                nc.vector.tensor_tensor(sq[:], og[:, h, :], og[:, h, :],
                                        OP.mult)
                ssq = pC.tile([1, TB], F32, tag="C")
                nc.tensor.matmul(ssq[:], onescol_sb[:], sq[:],
                                 start=True, stop=True)
                rinv2 = ogp.tile([1, TB], F32R, tag="rinv2", bufs=1)
                # rmsnorm with eps scaled by |q_t|^2: exactly compensates the
                # unnormalized q (output scales per-column by |q_t|).
                kh = h // (VH // KH)
                nc.vector.scalar_tensor_tensor(
                    rinv2[:], st["qsq"][0:1, kh, :], 1e-6 * DV, ssq[:],
                    OP.mult, OP.add)
                nc.scalar.activation(rinv2[:], rinv2[:], AF.Ln,
                                     scale=1.0 / DV, bias=eps12_sb[0:1])
                nc.scalar.activation(rinv2[:], rinv2[:], AF.Exp, scale=-0.5)
                rb = pC.tile([128, TB], F32, tag="C")
                nc.tensor.matmul(rb[:], onesrow_sb[:], rinv2[:],
                                 start=True, stop=True)
                nc.vector.scalar_tensor_tensor(
                    st["ogn"][:, h, :], og[:, h, :], normw_sb[:], rb[:],
                    OP.mult, OP.mult)
            for h in range(VH):
                groups["rms"].append(lambda h=h: u_rms(h))

            def u_outproj(c):
                rows = slice(tb * TB + c * L, tb * TB + (c + 1) * L)
                for co in range(C // TB):
                    ops2 = pC.tile([128, TB], F32, tag="C")
                    for h in range(VH):
                        nc.tensor.matmul(
                            ops2[:], st["ogn"][:, h, c * L:(c + 1) * L],
                            wout_sb[:, h, co * TB:(co + 1) * TB],
                            start=(h == 0), stop=(h == VH - 1))
                    ost = outp.tile([128, TB], BF16, tag="ost")
                    if co < 3:
                        nc.scalar.activation(ost[:], ops2[:], AF.Copy)
                    else:
                        nc.vector.tensor_copy(ost[:], ops2[:])
                    nc.gpsimd.dma_start(
                        out[b, rows, co * TB:(co + 1) * TB], ost[:])
            def u_rms_chunk(h, c):
                if h == 0 and c == 0:
                    st["ogn"] = ogp.tile([128, VH, TB], BF16, tag="ogn",
                                         bufs=1, name="ogn")
                    st["rinv2c"] = ogp.tile([1, TB], F32R, tag="rinv2",
                                            bufs=1, name="rinv2c")
                    st["sqc"] = ogp.tile([128, TB], BF16, tag="sq", bufs=2,
                                         name="sqc")
                og = st["og"]
                csl = slice(c * L, (c + 1) * L)
                sq = st["sqc"]
                nc.vector.tensor_tensor(sq[:, csl], og[:, h, csl],
                                        og[:, h, csl], OP.mult)
                ssq = pC.tile([1, TB], F32, tag="C")
                nc.tensor.matmul(ssq[0:1, 0:L], onescol_sb[:], sq[:, csl],
                                 start=True, stop=True)
                rinv2 = st["rinv2c"]
                kh = h // (VH // KH)
                nc.vector.scalar_tensor_tensor(
                    rinv2[0:1, csl], st["qsq"][0:1, kh, csl], 1e-6 * DV,
                    ssq[0:1, 0:L], OP.mult, OP.add)
                nc.scalar.activation(rinv2[0:1, csl], rinv2[0:1, csl], AF.Ln,
                                     scale=1.0 / DV, bias=eps12_sb[0:1])
                nc.scalar.activation(rinv2[0:1, csl], rinv2[0:1, csl],
                                     AF.Exp, scale=-0.5)
                rb = pC.tile([128, TB], F32, tag="C")
                nc.tensor.matmul(rb[:, 0:L], onesrow_sb[:], rinv2[0:1, csl],
                                 start=True, stop=True)
                nc.vector.scalar_tensor_tensor(
                    st["ogn"][:, h, csl], og[:, h, csl], normw_sb[:],
                    rb[:, 0:L], OP.mult, OP.mult)

            if tail:
                bunits = groups["B"]

                def u_tail(c):
                    bunits[c]()
                    for h in range(VH):
                        u_rms_chunk(h, c)
                    u_outproj(c)
                groups["B"] = [lambda c=c: u_tail(c) for c in range(NCH)]
                groups["rms"] = []
                return groups
            for c in range(NCH):
                groups["outproj"].append(lambda c=c: u_outproj(c))
            return groups

        # ---- scheduler: interleave A(i+1) with B/C(i) ----
        def run_step(A, BC):
            if A is None:
                for g in ("B", "rms", "outproj"):
                    for u in BC[g]:
                        u()
                return
            for u in A["head"]:
                u()
            P, C_, B_ = A["proj"], A["conv"], BC["B"]
            # proj/conv/B merged: conv starts as soon as its raw row-block is
            # copied, k/v conv tiles (which gate the chunkpre transposes)
            # before q; B units spread through for the S-chain.
            seq = [P[0], B_[0], P[1], P[2], C_[2], P[3], C_[3], B_[1],
                   P[4], C_[4], P[5], C_[5], B_[2], P[6], C_[6], P[7],
                   C_[7], P[12], B_[3], P[8], C_[0], P[9], C_[1],
                   P[10], P[11], P[13]]
            for u in seq:
                u()
            for u in BC["rms"]:
                u()
            for u in A["pre"]:
                u()
            for c in range(len(A["chunkpre"])):
                if c < len(BC["outproj"]):
                    BC["outproj"][c]()
                A["chunkpre"][c]()
            for u in BC["outproj"][len(A["chunkpre"]):]:
                u()

        A0 = emit_A_units(0)
        for g in ("head", "proj", "conv", "pre", "chunkpre"):
            for u in A0[g]:
                u()
        for i in range(NSLOT):
            a_next = emit_A_units(i + 1) if i + 1 < NSLOT else None
            run_step(a_next, emit_BC_units(i))

    return nc


_NC_CACHE = None
LAST_RESULT = None


def kernel(**inputs):
    global _NC_CACHE, LAST_RESULT
    x = np.asarray(inputs["x"], np.float32)
    input_pos = np.asarray(inputs["input_pos"])
    W_qkv = np.asarray(inputs["W_qkv"], np.float32)
    W_z = np.asarray(inputs["W_z"], np.float32)
    W_b = np.asarray(inputs["W_b"], np.float32)
    W_a = np.asarray(inputs["W_a"], np.float32)
    conv_w = np.asarray(inputs["conv_w"], np.float32)[:, 0, :]
    dt_bias = np.asarray(inputs["dt_bias"], np.float32)
    A_log = np.asarray(inputs["A_log"], np.float32)
    norm_w = np.asarray(inputs["norm_w"], np.float32)
    W_out = np.asarray(inputs["W_out"], np.float32)
    conv_state = np.asarray(inputs["conv_state"], np.float32)
    rec_state = np.asarray(inputs["recurrent_state"], np.float32)

    keep = 0.0 if int(input_pos[0]) == 0 else 1.0
    conv_state = conv_state * keep
    rec_state = rec_state * keep

    xt_host = np.ascontiguousarray(x.transpose(0, 2, 1)).astype(BF16_NP)

    in_maps = []
    for core in range(NCORES):
        vh = slice(VH * core, VH * (core + 1))
        qrows = slice(QCH * core, QCH * (core + 1))
        krows = slice(KEY + QCH * core, KEY + QCH * (core + 1))
        vrows = slice(2 * KEY + VCH * core, 2 * KEY + VCH * (core + 1))
        zrows = slice(ZCH * core, ZCH * (core + 1))

        Wt = np.concatenate(
            [W_qkv[qrows], W_qkv[krows], W_qkv[vrows], W_z[zrows],
             W_b[vh], np.zeros((32 - VH, C), np.float32),
             W_a[vh]], axis=0)                    # [TOTCH, C]
        wt_host = np.ascontiguousarray(Wt.T).astype(BF16_NP)      # [C, TOTCH]
        wout_host = np.ascontiguousarray(
            W_out[:, VCH * core:VCH * (core + 1)].T).astype(BF16_NP)

        cw = np.concatenate([conv_w[qrows], conv_w[krows], conv_w[vrows]], 0)
        convw_host = np.ascontiguousarray(
            cw.reshape(CONVCH // 128, 128, KCONV).transpose(1, 0, 2))

        cs = np.concatenate([conv_state[:, qrows], conv_state[:, krows],
                             conv_state[:, vrows]], axis=1)       # [B,1024,4]
        halo_host = np.ascontiguousarray(
            cs[:, :, 1:4].reshape(B, CONVCH // 128, 128, 3)
            .transpose(0, 2, 1, 3)).astype(BF16_NP)

        s0_host = np.ascontiguousarray(
            rec_state[:, vh].transpose(2, 0, 1, 3))   # [DK, B, VH, DV]
        dtb_host = np.ascontiguousarray(dt_bias[vh][:, None])
        nega_host = np.ascontiguousarray(-np.exp(A_log[vh])[:, None])
        normw_host = np.ascontiguousarray(norm_w[:, None])

        in_maps.append({
            "xt": xt_host, "wt": wt_host, "wout": wout_host,
            "convw": convw_host, "halo": halo_host, "s0": s0_host,
            "dtb": dtb_host, "nega": nega_host, "normw": normw_host,
        })

    if _NC_CACHE is None:
        _NC_CACHE = build_kernel()
    res = run_bass_kernel_spmd(_NC_CACHE, in_maps, core_ids=list(range(NCORES)))
    LAST_RESULT = res

    total = np.zeros((B, T, C), np.float32)
    for r in res.results:
        total += np.asarray(r["out"], np.float32)
    return total


# revision 32
# speedup vs baseline: 1.0250x; 1.0250x over previous
"""Trainium2 Bass kernel for nn_ExportGatedDeltaNet (gated linear attention
with depthwise conv, chunked recurrence).

Self-contained: hardcodes shapes/sharding. Sharding: 8-way tensor-parallel
over heads (each core owns 4 of the 32 value heads / 2 of the 16 key heads);
both batch elements are processed on every core (slots interleave the two
batches). Each core computes a full [B, T, C] partial of the output
projection over its head slice; the host sums the 8 partials.

v2 redesign vs baseline:
- q is left unnormalized: the gated RMSNorm downstream is invariant to
  per-timestep scaling of the attention output, so q/|q| is unnecessary.
- k is normalized in transposed space (per-partition row scale after the
  PE transpose) -- no broadcast matmuls, no DVE reciprocal.
- rsqrt/log-sigmoid/softplus are computed as exp/ln pairs so every
  non-silu activation lives in the single natural_log_exp table
  (2 act-table loads per slot instead of ~14).
- DMA transposes (1.2us each on SP) replaced by PE transposes + Pool copies.
- Emission interleaves slot s's recurrence (phase B/C) with slot s+1's
  projection/conv/decay precompute (phase A) and alternates batch elements
  between consecutive slots, keeping the PE stream dense (p-state ramp).
"""

import numpy as np
import ml_dtypes

import concourse.bass as bass
import concourse.tile as tile
from concourse import mybir
from concourse.vector_clock import ScopedClock, VectorClock
from concourse.bass_utils import run_bass_kernel_spmd

F32 = mybir.dt.float32
F32R = mybir.dt.float32r
BF16 = mybir.dt.bfloat16
AF = mybir.ActivationFunctionType
OP = mybir.AluOpType
BF16_NP = ml_dtypes.bfloat16

NK, NV, DK, DV, KCONV, C = 16, 32, 128, 128, 4, 2048
KEY = NK * DK            # 2048
B, T = 2, 2048
L = 128                  # recurrence chunk length
TB = 512                 # t-block (slot length)
NTB = T // TB            # 4
NCH = TB // L            # chunks per slot
NCORES = 8

# per-core head slice
VH = NV // NCORES        # 4 value heads
KH = NK // NCORES        # 2 key heads
QCH = KH * DK            # 256
VCH = VH * DV            # 512
ZCH = VH * DV            # 512
CONVCH = 2 * QCH + VCH   # 1024 channels through the conv
TOTCH = CONVCH + ZCH + 32 + VH  # 1572: ..., b(4), pad(28), a(4)
N_CT = C // 128          # 16 contraction tiles
N_CONVT = CONVCH // 128  # 8
N_ZT = ZCH // 128        # 4
N_WT = TOTCH // 128      # 12 full tiles + 36 extra cols handled separately


def _walrus_safe_drain(self, tick_clock, wait_clock):
    # The container's walrus rejects >1 sync-wait on CTRL-class instructions;
    # split the final drain's waits across single-wait nops.
    vals = eval(repr(tick_clock.global_clock).replace("VectorClock", ""))
    for j, v in enumerate(vals):
        if not v:
            continue
        masked = [0] * len(vals)
        masked[j] = v
        nop_inst = self.nc.sync.nop(nofuse=True)
        wait_clock.add_sem_waits(
            nop_inst.ins, ScopedClock({None: VectorClock(masked)})
        )
    self.nc.sync.drain()
    self.nc.all_engine_barrier()
    popped = self.nc._tile_sem_poison_stack.pop()
    assert popped is self._sem_poison
    self.nc.clear_and_free_semaphores(list(self.sems.allocated().values()))
    self.nc.all_engine_barrier()


tile.TileContext._drain_and_barrier = _walrus_safe_drain


# The container's walrus rejects >1 sync-wait on any instruction. Tile's
# semaphore pass emits multi-wait instructions, so split them at the BIR-JSON
# level: hoist all but one wait onto NoOps (same engine) inserted just before.
_orig_to_json_bytes = bass.Bass.to_json_bytes
_WSPLIT = [0]


def _split_multi_waits(self, *args, **kwargs):
    import json
    raw = _orig_to_json_bytes(self, *args, **kwargs)
    m = json.loads(raw)
    changed = False
    for f in m["functions"]:
        for bb in f["blocks"]:
            out_insts = []
            for inst in bb["instructions"]:
                si = inst.get("sync_info")
                waits = (si or {}).get("on_wait") or []
                if len(waits) > 1:
                    changed = True
                    for w in waits[:-1]:
                        _WSPLIT[0] += 1
                        out_insts.append({
                            "debug": inst.get("debug"),
                            "engine": inst["engine"],
                            "ins": [], "outs": [],
                            "name": f"I-wsplit-{_WSPLIT[0]}",
                            "opcode": "NoOp",
                            "sync_info": {"on_update": [], "on_wait": [w]},
                        })
                    si["on_wait"] = [waits[-1]]
                out_insts.append(inst)
            bb["instructions"] = out_insts
    if not changed:
        return raw
    return json.dumps(m).encode()


bass.Bass.to_json_bytes = _split_multi_waits

# HWDGE DMAs execute on DMA-queue timelines, where a hoisted same-engine NoOp
# wait does not gate them. Route static DMAs through the SP sequencer instead
# so program order (and the NoOp wait splitting) applies to them too.
import concourse.bass_utils as _bu

_orig_run_command = _bu.run_command


def _patched_run_command(argv, **kwargs):
    argv = [a.replace("--assign-static-dmas-to-sp=false",
                      "--assign-static-dmas-to-sp=true")
            for a in argv]
    return _orig_run_command(argv, **kwargs)


_bu.run_command = _patched_run_command


def build_kernel():
    nc = bass.Bass(num_swdge_queues=4)

    xt = nc.dram_tensor("xt", [B, C, T], BF16, kind="ExternalInput")
    wt = nc.dram_tensor("wt", [C, TOTCH], BF16, kind="ExternalInput")
    wout = nc.dram_tensor("wout", [VCH, C], BF16, kind="ExternalInput")
    convw = nc.dram_tensor("convw", [128, N_CONVT, KCONV], F32,
                           kind="ExternalInput")
    halo = nc.dram_tensor("halo", [B, 128, N_CONVT, KCONV - 1], BF16,
                          kind="ExternalInput")
    s0 = nc.dram_tensor("s0", [DK, B, VH, DV], F32, kind="ExternalInput")
    dtb = nc.dram_tensor("dtb", [VH, 1], F32, kind="ExternalInput")
    nega = nc.dram_tensor("nega", [VH, 1], F32, kind="ExternalInput")
    normw = nc.dram_tensor("normw", [128, 1], F32, kind="ExternalInput")
    out = nc.dram_tensor("out", [B, T, C], BF16, kind="ExternalOutput")

    # constants embedded in the NEFF
    ut_np = np.triu(np.ones((L, L), np.float32))              # [u,t]: u<=t
    UT = nc.inline_tensor(ut_np.astype(BF16_NP), name="UT")
    STA = nc.inline_tensor((1.0 - ut_np).astype(BF16_NP), name="STA")  # u>t'
    ONESM = nc.inline_tensor(np.ones((L, L), BF16_NP), name="ONESM")
    NEGM = nc.inline_tensor(
        np.where(ut_np > 0, 0.0, -1e30).astype(np.float32), name="NEGM")
    IDENT128 = nc.inline_tensor(np.eye(128, dtype=BF16_NP), name="IDENT128")
    ONES_COL = nc.inline_tensor(np.ones((128, 1), BF16_NP), name="ONES_COL")
    ONE_F = nc.inline_tensor(np.ones((128, 1), np.float32), name="ONE_F")
    EPS12 = nc.inline_tensor(np.full((128, 1), 1e-12, np.float32), name="EPS12")
    EPS6 = nc.inline_tensor(np.full((128, 1), 1e-6, np.float32), name="EPS6")
    ONES_ROW = nc.inline_tensor(np.ones((1, 128), np.float32), name="ONES_ROW")

    from contextlib import ExitStack
    with nc.allow_low_precision(reason="bf16 compute by design"), \
         tile.TileContext(nc) as tc, ExitStack() as stack:
        consts = stack.enter_context(tc.tile_pool(name="consts", bufs=1))
        wpool = stack.enter_context(tc.tile_pool(name="wpool", bufs=1))
        statep = stack.enter_context(tc.tile_pool(name="statep", bufs=1))
        xpool = stack.enter_context(tc.tile_pool(name="xpool", bufs=2))
        rawp = stack.enter_context(tc.tile_pool(name="rawp", bufs=2))
        projp = stack.enter_context(tc.tile_pool(name="projp", bufs=1))
        decp = stack.enter_context(tc.tile_pool(name="decp", bufs=2))
        ogp = stack.enter_context(tc.tile_pool(name="ogp", bufs=2))
        outp = stack.enter_context(tc.tile_pool(name="outp", bufs=2))
        pA = stack.enter_context(tc.tile_pool(name="pA", bufs=2, space="PSUM"))
        pTR = stack.enter_context(tc.tile_pool(name="pTR", bufs=2, space="PSUM"))
        pB = stack.enter_context(tc.tile_pool(name="pB", bufs=2, space="PSUM"))
        pC = stack.enter_context(tc.tile_pool(name="pC", bufs=2, space="PSUM"))


        # ---- per-slot persistent-ish tiles are allocated inside units ----
        slots = [(i % 2, i // 2) for i in range(B * NTB)]  # (b, tb) interleaved
        NSLOT = len(slots)

        # cross-unit state passed via dicts keyed by slot index
        ST = {i: {} for i in range(NSLOT)}

        # resident weights (xt0 is loaded first, below, so proj can start)
        wt_sb = wpool.tile([128, N_CT, TOTCH], BF16)
        wout_sb = wpool.tile([128, VH, C], BF16)

        # slot 0's x is loaded up front
        xt0 = xpool.tile([128, N_CT, TB], BF16, tag="xt")
        ST[0]["xt"] = xt0
        b0, tb0 = slots[0]
        nc.gpsimd.dma_start(
            xt0[:],
            xt[b0].rearrange("(ko p) t -> p ko t", p=128)
            [:, :, tb0 * TB:(tb0 + 1) * TB])
        wt_r = wt.rearrange("(ko p) f -> p ko f", p=128)
        for c0, c1 in [(0, 128), (128, 384), (384, 768), (768, 1152),
                       (1152, TOTCH)]:
            nc.gpsimd.dma_start(wt_sb[:, :, c0:c1], wt_r[:, :, c0:c1])
        nc.gpsimd.dma_start(wout_sb[:],
                            wout.rearrange("(vo p) f -> p vo f", p=128))

        # ---- load constants ----
        ut_sb = consts.tile([L, L], BF16)
        nc.gpsimd.dma_start(ut_sb[:], UT[:])
        sta_sb = consts.tile([L, L], BF16)
        nc.gpsimd.dma_start(sta_sb[:], STA[:])
        onesm_sb = consts.tile([L, L], BF16)
        nc.gpsimd.dma_start(onesm_sb[:], ONESM[:])
        negm_sb = consts.tile([L, L], F32)
        nc.gpsimd.dma_start(negm_sb[:], NEGM[:])
        ident128_sb = consts.tile([128, 128], BF16)
        nc.gpsimd.dma_start(ident128_sb[:], IDENT128[:])
        onescol_sb = consts.tile([128, 1], BF16)
        nc.gpsimd.dma_start(onescol_sb[:], ONES_COL[:])
        one_sb = consts.tile([128, 1], F32)
        nc.gpsimd.dma_start(one_sb[:], ONE_F[:])
        eps12_sb = consts.tile([128, 1], F32)
        nc.gpsimd.dma_start(eps12_sb[:], EPS12[:])
        eps6_sb = consts.tile([128, 1], F32)
        nc.gpsimd.dma_start(eps6_sb[:], EPS6[:])
        onesrow_sb = consts.tile([1, 128], F32R)
        nc.gpsimd.dma_start(onesrow_sb[:], ONES_ROW[:].bitcast(F32R))
        convw_sb = consts.tile([128, N_CONVT, KCONV], F32)
        nc.gpsimd.dma_start(convw_sb[:], convw[:])
        dtb_sb = consts.tile([VH, 1], F32)
        nc.gpsimd.dma_start(dtb_sb[:], dtb[:])
        nega_sb = consts.tile([VH, 1], F32)
        nc.gpsimd.dma_start(nega_sb[:], nega[:])
        normw_sb = consts.tile([128, 1], F32)
        nc.gpsimd.dma_start(normw_sb[:], normw[:])

        # recurrent state (f32 truth + bf16 matmul copy) and conv halo
        S = statep.tile([128, B, VH, DV], F32, tag="S")
        nc.gpsimd.dma_start(S[:], s0[:])
        Sb = statep.tile([128, B, VH, DV], BF16, tag="Sb")
        nc.vector.tensor_copy(Sb[:], S[:])
        halo_sb = statep.tile([128, B, N_CONVT, KCONV - 1], BF16, tag="halo")
        nc.gpsimd.dma_start(halo_sb[:], halo.rearrange("b p k j -> p b k j"))

        def emit_A_units(i):
            """Phase A for slot i: projection, conv, decay precompute.
            Returns a list of callables."""
            b, tb = slots[i]
            tsl = slice(tb * TB, (tb + 1) * TB)
            st = ST[i]
            units = []

            def u_prefetch():
                # prefetch x for the *next* slot; A(i) is emitted one step
                # early, so this lands ~1.5 steps before the data is consumed.
                if i + 1 < NSLOT:
                    nb_, ntb_ = slots[i + 1]
                    xt_t = xpool.tile([128, N_CT, TB], BF16, tag="xt")
                    ST[i + 1]["xt"] = xt_t
                    nc.gpsimd.dma_start(
                        xt_t[:],
                        xt[nb_].rearrange("(ko p) t -> p ko t", p=128)
                        [:, :, ntb_ * TB:(ntb_ + 1) * TB])
            units.append(u_prefetch)

            def u_rawinit():
                raw = rawp.tile([128, N_CONVT, TB + KCONV - 1], BF16, tag="raw")
                st["raw"] = raw
                nc.gpsimd.tensor_copy(raw[:, :, 0:KCONV - 1], halo_sb[:, b])
                st["z"] = projp.tile([128, N_ZT, TB], BF16, tag="z", bufs=2,
                                     name="z")
                st["qf"] = projp.tile([128, KH, TB], BF16, tag="qf", name="qf")
                st["kf"] = projp.tile([128, KH, TB], BF16, tag="kf", name="kf")
                st["vf"] = projp.tile([128, VH, TB], BF16, tag="vf", name="vf")
            units.append(u_rawinit)

            def u_proj(cht):
                ps = pA.tile([128, TB], F32, tag="A")
                for ct in range(N_CT):
                    nc.tensor.matmul(
                        ps[:], wt_sb[:, ct, cht * 128:(cht + 1) * 128],
                        st["xt"][:, ct, :],
                        start=(ct == 0), stop=(ct == N_CT - 1))
                if cht < N_CONVT:
                    if cht % 2 == 0:
                        nc.vector.tensor_copy(
                            st["raw"][:, cht, KCONV - 1:TB + KCONV - 1], ps[:])
                    else:
                        nc.scalar.activation(
                            st["raw"][:, cht, KCONV - 1:TB + KCONV - 1], ps[:],
                            AF.Copy)
                else:
                    zi = cht - N_CONVT
                    nc.scalar.activation(st["z"][:, zi, :], ps[:], AF.Silu)
            for cht in range(N_WT):
                units.append(lambda cht=cht: u_proj(cht))

            def u_halo_save():
                # stash the last 3 input positions for this batch's next slot
                nc.gpsimd.tensor_copy(
                    halo_sb[:, b], st["raw"][:, :, TB:TB + KCONV - 1])
            units.append(u_halo_save)

            def u_ba():
                ps = pA.tile([128, TB], F32, tag="A")
                for ct in range(N_CT):
                    nc.tensor.matmul(
                        ps[0:32 + VH, :], wt_sb[:, ct, N_WT * 128:TOTCH],
                        st["xt"][:, ct, :],
                        start=(ct == 0), stop=(ct == N_CT - 1))
                gsp = projp.tile([VH, 2, TB], BF16, tag="gsp")
                st["gsp"] = gsp
                tmp = projp.tile([VH, TB], BF16, tag="batmp", bufs=2)
                # lnb' = -ln(sigmoid(b)) = ln(1 + exp(-b))
                nc.scalar.activation(tmp[:], ps[0:VH, :], AF.Exp, scale=-1.0)
                nc.scalar.activation(gsp[:, 1, :], tmp[:], AF.Ln, bias=one_sb[0:VH])
                # softplus(a + dtb) = ln(1 + exp(a + dtb))
                tmp2 = projp.tile([VH, TB], BF16, tag="batmp", bufs=2)
                nc.scalar.activation(tmp2[:], ps[32:32 + VH, :], AF.Exp,
                                     bias=dtb_sb[:])
                tmp3 = projp.tile([VH, TB], BF16, tag="batmp2", bufs=2)
                nc.scalar.activation(tmp3[:], tmp2[:], AF.Ln, bias=one_sb[0:VH])
                # g = -exp(A_log) * softplus
                nc.vector.tensor_scalar(gsp[:, 0, :], tmp3[:], nega_sb[:],
                                        None, OP.mult)
            units.append(u_ba)

            def u_conv(cht):
                acc = projp.tile([128, TB], BF16, tag="convacc", bufs=2)
                nc.vector.tensor_scalar(
                    acc[:], st["raw"][:, cht, 0:TB], convw_sb[:, cht, 0:1],
                    None, OP.mult)
                for j in range(1, KCONV):
                    nc.vector.scalar_tensor_tensor(
                        acc[:], st["raw"][:, cht, j:TB + j],
                        convw_sb[:, cht, j:j + 1], acc[:],
                        OP.mult, OP.add)
                if cht < KH:
                    nc.scalar.activation(st["qf"][:, cht, :], acc[:], AF.Silu)
                elif cht < 2 * KH:
                    nc.scalar.activation(st["kf"][:, cht - KH, :], acc[:],
                                         AF.Silu)
                else:
                    nc.scalar.activation(st["vf"][:, cht - 2 * KH, :], acc[:],
                                         AF.Silu)
            for cht in range(N_CONVT):
                units.append(lambda cht=cht: u_conv(cht))

            def u_qsq():
                # ssq of raw q per timestep, for exact rmsnorm eps compensation
                qsq = decp.tile([1, KH, TB], BF16, tag="qsq", name="qsq")
                st["qsq"] = qsq
                for kh in range(KH):
                    sqq = ogp.tile([128, TB], BF16, tag="sq", bufs=2)
                    nc.vector.tensor_tensor(sqq[:], st["qf"][:, kh, :],
                                            st["qf"][:, kh, :], OP.mult)
                    qsp = pC.tile([1, TB], F32, tag="C")
                    nc.tensor.matmul(qsp[:], onescol_sb[:], sqq[:],
                                     start=True, stop=True)
                    nc.scalar.activation(qsq[0:1, kh, :], qsp[:], AF.Copy)
            units.append(u_qsq)

            def u_predecl():
                st["gspT"] = decp.tile([128, NCH, 2 * VH], F32, tag="gspT",
                                       bufs=1, name="gspT")
                st["ktT"] = decp.tile([128, NCH, KH, L], BF16, tag="ktT", name="ktT")
                st["vT"] = decp.tile([128, NCH, VH, L], BF16, tag="vT", name="vT")
                st["qh"] = decp.tile([128, NCH, VH, L], BF16, tag="qh", name="qh")
                st["PT"] = decp.tile([128, NCH, VH, L], BF16, tag="PT", name="PT")
                st["ksq"] = decp.tile([128, NCH * KH], F32, tag="ksq", name="ksq")
                st["rinvk"] = decp.tile([128, NCH, KH], F32, tag="rinvk", name="rinvk")
                st["rk2"] = decp.tile([128, NCH * VH], F32, tag="rk2", name="rk2")
                st["dtlast"] = decp.tile([128, NCH * VH], F32, tag="dtlast", name="dtlast")
                st["dclast"] = decp.tile([128, NCH * VH], F32, tag="dclast", name="dclast")
            units.append(u_predecl)

            def u_chunkpre(c):
                t0 = c * L
                csl = slice(t0, t0 + L)
                # --- transposes: k, v chunks -> t-partitioned (PE + Pool) ---
                trq = pTR.tile([128, 6 * L + 2 * VH], BF16, tag="trb")
                for kh in range(KH):
                    nc.tensor.transpose(trq[:, kh * L:(kh + 1) * L],
                                        st["kf"][:, kh, csl], ident128_sb[:])
                for h in range(VH):
                    nc.tensor.transpose(
                        trq[:, (KH + h) * L:(KH + h + 1) * L],
                        st["vf"][:, h, csl], ident128_sb[:])
                nc.scalar.activation(
                    st["ktT"][:, c].rearrange("p a b -> p (a b)"),
                    trq[:, 0:KH * L], AF.Copy)
                nc.vector.tensor_copy(st["vT"][:, c],
                                      trq[:, KH * L:(KH + VH) * L])
                # --- gsp transposes ride in the same bf16 quad ---
                nc.tensor.transpose(trq[:, 6 * L:6 * L + VH],
                                    st["gsp"][0:VH, 0, csl],
                                    ident128_sb[0:VH, 0:VH])
                nc.tensor.transpose(trq[:, 6 * L + VH:6 * L + 2 * VH],
                                    st["gsp"][0:VH, 1, csl],
                                    ident128_sb[0:VH, 0:VH])
                nc.scalar.activation(st["gspT"][:, c],
                                      trq[:, 6 * L:6 * L + 2 * VH], AF.Copy)
                quad = pB.tile([128, TB], F32, tag="B")
                # --- Ball / Dps / dtps ---
                ball = decp.tile([128, VH, L], BF16, tag="ball")
                for h in range(VH):
                    nc.vector.tensor_scalar(
                        ball[:, h, :], ut_sb[:], st["gspT"][:, c, h:h + 1],
                        None, OP.mult)
                dps = pA.tile([128, VH, L], F32, tag="A")
                nc.tensor.matmul(dps[:].rearrange("p a b -> p (a b)"),
                                 sta_sb[:],
                                 ball[:].rearrange("p a b -> p (a b)"),
                                 start=True, stop=True)
                dtps = pA.tile([128, VH, L], F32, tag="A")
                nc.tensor.matmul(dtps[:].rearrange("p a b -> p (a b)"),
                                 onesm_sb[:],
                                 ball[:].rearrange("p a b -> p (a b)"),
                                 start=True, stop=True)
                # ebr = exp(cumsum g) broadcast; last col kept in f32
                ebr = decp.tile([128, VH, L], BF16, tag="ebr")
                nc.scalar.activation(ebr[:].rearrange("p a b -> p (a b)"),
                                     dtps[:].rearrange("p a b -> p (a b)"),
                                     AF.Exp)
                nc.scalar.activation(
                    st["dtlast"][:, c * VH:(c + 1) * VH],
                    dtps[:, :, L - 1:L].rearrange("p a b -> p (a b)"), AF.Exp)
                # Eall = Dps - lnb' + negm ; Decay = exp(Eall)
                escr = decp.tile([128, VH, L], BF16, tag="escr")
                for h in range(VH):
                    nc.vector.scalar_tensor_tensor(
                        escr[:, h, :], dps[:, h, :],
                        st["gspT"][:, c, VH + h:VH + h + 1], negm_sb[:],
                        OP.subtract, OP.add)
                decay = decp.tile([128, VH, L], BF16, tag="decay")
                nc.scalar.activation(decay[:].rearrange("p a b -> p (a b)"),
                                     escr[:].rearrange("p a b -> p (a b)"),
                                     AF.Exp)
                nc.scalar.activation(
                    st["dclast"][:, c * VH:(c + 1) * VH],
                    escr[:, :, L - 1:L].rearrange("p a b -> p (a b)"), AF.Exp)
                # --- k norm in transposed space ---
                kscr = decp.tile([128, L], BF16, tag="kscr")
                for kh in range(KH):
                    nc.vector.scalar_tensor_tensor(
                        kscr[:], st["ktT"][:, c, kh], 1.0,
                        st["ktT"][:, c, kh], OP.mult, OP.mult,
                        accum_out=st["ksq"][:, c * KH + kh:c * KH + kh + 1])
                nc.scalar.activation(st["rinvk"][:, c],
                                     st["ksq"][:, c * KH:(c + 1) * KH],
                                     AF.Ln, bias=eps12_sb[:])
                nc.scalar.activation(st["rinvk"][:, c], st["rinvk"][:, c],
                                     AF.Exp, scale=-0.5)
                # --- Pps (raw attention logits), PT, qh, rk2 ---
                for kh in range(KH):
                    nc.tensor.matmul(quad[:, kh * L:(kh + 1) * L],
                                     st["kf"][:, kh, csl],
                                     st["qf"][:, kh, csl],
                                     start=True, stop=True)
                for h in range(VH):
                    kh = h // (VH // KH)
                    nc.vector.scalar_tensor_tensor(
                        st["PT"][:, c, h], quad[:, kh * L:(kh + 1) * L],
                        st["rinvk"][:, c, kh:kh + 1], decay[:, h, :],
                        OP.mult, OP.mult)
                    nc.vector.tensor_tensor(
                        st["qh"][:, c, h], st["qf"][:, kh, csl], ebr[:, h, :],
                        OP.mult)
                    nc.vector.tensor_tensor(
                        st["rk2"][:, c * VH + h:c * VH + h + 1],
                        st["rinvk"][:, c, kh:kh + 1],
                        st["dclast"][:, c * VH + h:c * VH + h + 1], OP.mult)
            groups = {"head": units[0:2], "proj": units[2:2 + N_WT + 2],
                      "conv": units[2 + N_WT + 2:2 + N_WT + 2 + N_CONVT],
                      "pre": units[2 + N_WT + 2 + N_CONVT:],
                      "chunkpre": [lambda c=c: u_chunkpre(c)
                                   for c in range(NCH)]}
            return groups

        def emit_BC_units(i):
            """Phase B (recurrence) + C (rmsnorm + out-proj) for slot i."""
            b, tb = slots[i]
            st = ST[i]
            units = []

            def u_B(c):
                og = st["og"]
                kt2 = decp.tile([128, VH, L], BF16, tag="kt2")
                for h in range(VH):
                    nc.vector.tensor_scalar(
                        kt2[:, h, :], st["ktT"][:, c, h // (VH // KH)],
                        st["rk2"][:, c * VH + h:c * VH + h + 1], None, OP.mult)
                pops = pB.tile([128, TB], F32, tag="B")
                sps = pB.tile([128, TB], F32, tag="B")
                for h in range(VH):
                    nc.tensor.matmul(pops[:, h * L:(h + 1) * L],
                                     Sb[:, b, h, :], st["qh"][:, c, h],
                                     start=True, stop=False)
                    nc.tensor.matmul(sps[:, h * L:(h + 1) * L],
                                     kt2[:, h, :], st["vT"][:, c, h],
                                     start=True, stop=True)
                    nc.tensor.matmul(pops[:, h * L:(h + 1) * L],
                                     st["vT"][:, c, h], st["PT"][:, c, h],
                                     start=False, stop=True)
                for h in range(VH):
                    nc.vector.tensor_tensor(
                        og[:, h, c * L:(c + 1) * L],
                        pops[:, h * L:(h + 1) * L],
                        st["z"][:, h, c * L:(c + 1) * L], OP.mult)
                for h in range(VH):
                    nc.vector.scalar_tensor_tensor(
                        S[:, b, h, :], S[:, b, h, :],
                        st["dtlast"][:, c * VH + h:c * VH + h + 1],
                        sps[:, h * L:(h + 1) * L], OP.mult, OP.add)
                nc.scalar.activation(
                    Sb[:, b].rearrange("p a b -> p (a b)"),
                    S[:, b].rearrange("p a b -> p (a b)"), AF.Copy)

            def u_B0(c=0):
                st["og"] = ogp.tile([128, VH, TB], BF16, tag="og", name="og")
                u_B(0)
            units.append(u_B0)
            for c in range(1, NCH):
                units.append(lambda c=c: u_B(c))

            groups = {"B": list(units), "rms": [], "outproj": []}

            def u_rms(h):
                if h == 0:
                    st["ogn"] = ogp.tile([128, VH, TB], BF16, tag="ogn",
                                         bufs=1, name="ogn")
                og = st["og"]
                sq = ogp.tile([128, TB], BF16, tag="sq", bufs=2)
                nc.vector.tensor_tensor(sq[:], og[:, h, :], og[:, h, :],
                                        OP.mult)
                ssq = pC.tile([1, TB], F32, tag="C")
                nc.tensor.matmul(ssq[:], onescol_sb[:], sq[:],
                                 start=True, stop=True)
                rinv2 = ogp.tile([1, TB], F32R, tag="rinv2", bufs=1)
                # rmsnorm with eps scaled by |q_t|^2: exactly compensates the
                # unnormalized q (output scales per-column by |q_t|).
                kh = h // (VH // KH)
                nc.vector.scalar_tensor_tensor(
                    rinv2[:], st["qsq"][0:1, kh, :], 1e-6 * DV, ssq[:],
                    OP.mult, OP.add)
                nc.scalar.activation(rinv2[:], rinv2[:], AF.Ln,
                                     scale=1.0 / DV, bias=eps12_sb[0:1])
                nc.scalar.activation(rinv2[:], rinv2[:], AF.Exp, scale=-0.5)
                rb = pC.tile([128, TB], F32, tag="C")
                nc.tensor.matmul(rb[:], onesrow_sb[:], rinv2[:],
                                 start=True, stop=True)
                nc.vector.scalar_tensor_tensor(
                    st["ogn"][:, h, :], og[:, h, :], normw_sb[:], rb[:],
                    OP.mult, OP.mult)
            for h in range(VH):
                groups["rms"].append(lambda h=h: u_rms(h))

            def u_outproj(c):
                rows = slice(tb * TB + c * L, tb * TB + (c + 1) * L)
                for co in range(C // TB):
                    ops2 = pC.tile([128, TB], F32, tag="C")
                    for h in range(VH):
                        nc.tensor.matmul(
                            ops2[:], st["ogn"][:, h, c * L:(c + 1) * L],
                            wout_sb[:, h, co * TB:(co + 1) * TB],
                            start=(h == 0), stop=(h == VH - 1))
                    ost = outp.tile([128, TB], BF16, tag="ost")
                    if co < 3:
                        nc.scalar.activation(ost[:], ops2[:], AF.Copy)
                    else:
                        nc.vector.tensor_copy(ost[:], ops2[:])
                    nc.gpsimd.dma_start(
                        out[b, rows, co * TB:(co + 1) * TB], ost[:])
            for c in range(NCH):
                groups["outproj"].append(lambda c=c: u_outproj(c))
            return groups

        # ---- scheduler: interleave A(i+1) with B/C(i) ----
        def run_step(A, BC):
            if A is None:
                for g in ("B", "rms", "outproj"):
                    for u in BC[g]:
                        u()
                return
            for u in A["head"]:
                u()
            P, C_, B_ = A["proj"], A["conv"], BC["B"]
            # proj/conv/B merged: conv starts as soon as its raw row-block is
            # copied, k/v conv tiles (which gate the chunkpre transposes)
            # before q; B units spread through for the S-chain.
            seq = [P[0], B_[0], P[1], P[2], C_[2], P[3], C_[3], B_[1],
                   P[4], C_[4], P[5], C_[5], B_[2], P[6], C_[6], P[7],
                   C_[7], P[12], B_[3], P[8], C_[0], P[9], C_[1],
                   P[10], P[11], P[13]]
            for u in seq:
                u()
            for u in BC["rms"]:
                u()
            for u in A["pre"]:
                u()
            for c in range(len(A["chunkpre"])):
                if c < len(BC["outproj"]):
                    BC["outproj"][c]()
                A["chunkpre"][c]()
            for u in BC["outproj"][len(A["chunkpre"]):]:
                u()

        A0 = emit_A_units(0)
        for g in ("head", "proj", "conv", "pre", "chunkpre"):
            for u in A0[g]:
                u()
        for i in range(NSLOT):
            a_next = emit_A_units(i + 1) if i + 1 < NSLOT else None
            run_step(a_next, emit_BC_units(i))

    return nc


_NC_CACHE = None
LAST_RESULT = None


def kernel(**inputs):
    global _NC_CACHE, LAST_RESULT
    x = np.asarray(inputs["x"], np.float32)
    input_pos = np.asarray(inputs["input_pos"])
    W_qkv = np.asarray(inputs["W_qkv"], np.float32)
    W_z = np.asarray(inputs["W_z"], np.float32)
    W_b = np.asarray(inputs["W_b"], np.float32)
    W_a = np.asarray(inputs["W_a"], np.float32)
    conv_w = np.asarray(inputs["conv_w"], np.float32)[:, 0, :]
    dt_bias = np.asarray(inputs["dt_bias"], np.float32)
    A_log = np.asarray(inputs["A_log"], np.float32)
    norm_w = np.asarray(inputs["norm_w"], np.float32)
    W_out = np.asarray(inputs["W_out"], np.float32)
    conv_state = np.asarray(inputs["conv_state"], np.float32)
    rec_state = np.asarray(inputs["recurrent_state"], np.float32)

    keep = 0.0 if int(input_pos[0]) == 0 else 1.0
    conv_state = conv_state * keep
    rec_state = rec_state * keep

    xt_host = np.ascontiguousarray(x.transpose(0, 2, 1)).astype(BF16_NP)

    in_maps = []
    for core in range(NCORES):
        vh = slice(VH * core, VH * (core + 1))
        qrows = slice(QCH * core, QCH * (core + 1))
        krows = slice(KEY + QCH * core, KEY + QCH * (core + 1))
        vrows = slice(2 * KEY + VCH * core, 2 * KEY + VCH * (core + 1))
        zrows = slice(ZCH * core, ZCH * (core + 1))

        Wt = np.concatenate(
            [W_qkv[qrows], W_qkv[krows], W_qkv[vrows], W_z[zrows],
             W_b[vh], np.zeros((32 - VH, C), np.float32),
             W_a[vh]], axis=0)                    # [TOTCH, C]
        wt_host = np.ascontiguousarray(Wt.T).astype(BF16_NP)      # [C, TOTCH]
        wout_host = np.ascontiguousarray(
            W_out[:, VCH * core:VCH * (core + 1)].T).astype(BF16_NP)

        cw = np.concatenate([conv_w[qrows], conv_w[krows], conv_w[vrows]], 0)
        convw_host = np.ascontiguousarray(
            cw.reshape(CONVCH // 128, 128, KCONV).transpose(1, 0, 2))

        cs = np.concatenate([conv_state[:, qrows], conv_state[:, krows],
                             conv_state[:, vrows]], axis=1)       # [B,1024,4]
        halo_host = np.ascontiguousarray(
            cs[:, :, 1:4].reshape(B, CONVCH // 128, 128, 3)
            .transpose(0, 2, 1, 3)).astype(BF16_NP)

        s0_host = np.ascontiguousarray(
            rec_state[:, vh].transpose(2, 0, 1, 3))   # [DK, B, VH, DV]
        dtb_host = np.ascontiguousarray(dt_bias[vh][:, None])
        nega_host = np.ascontiguousarray(-np.exp(A_log[vh])[:, None])
        normw_host = np.ascontiguousarray(norm_w[:, None])

        in_maps.append({
            "xt": xt_host, "wt": wt_host, "wout": wout_host,
            "convw": convw_host, "halo": halo_host, "s0": s0_host,
            "dtb": dtb_host, "nega": nega_host, "normw": normw_host,
        })

    if _NC_CACHE is None:
        _NC_CACHE = build_kernel()
    res = run_bass_kernel_spmd(_NC_CACHE, in_maps, core_ids=list(range(NCORES)))
    LAST_RESULT = res

    total = np.zeros((B, T, C), np.float32)
    for r in res.results:
        total += np.asarray(r["out"], np.float32)
    return total
